# revision 26
# baseline (speedup 1.0000x reference)
"""DISCO S2 conv (DiscreteContinuousConvS2) Trainium2 Bass kernel — v2.

Spectral algorithm (per core; 8 cores = batch(4) x C_out-half(2)):
  1. einsum over C_in (stage A):  xw[po, la, (k,o)] = x[:, la, po].T @ w2
  2. forward rDFT over longitude (stage B), psum -> xh[f, (k,o), la] bf16
  3. spectral multiply-accumulate over 10 nonzero (k,dla) pairs on the
     Vector engine in bf16 (2x DVE mode; parity-aligned dual xh copies)
  4. inverse rDFT (stage E) with dinv stationary, out [po, o, ho] layout
All matmul inputs bf16 (fp32 PSUM accumulate).  Latitude processed in
6 ho-bands with +-4 la halo; copies on Scalar engine, xh shift on GPSIMD.
"""
import sys
import numpy as np

for _p in ("/opt/trn_rl_repo",):
    if _p not in sys.path:
        sys.path.insert(0, _p)

import ml_dtypes
BF16 = ml_dtypes.bfloat16

NLAT, NLON, NF, FDIM = 181, 360, 181, 362
K, B, CIN, COUT, OH = 2, 4, 96, 96, 48
FS = [(0, 128), (128, 256), (256, 362)]
PS = [(0, 128), (128, 256), (256, 360)]
BANDS = [(0, 31), (31, 62), (62, 93), (93, 124), (124, 155), (155, 181)]
PHW = 192                      # padded per-pair phat width (>= 181, covers +W reads)
NZ = [(1, 0), (0, -1), (0, 0), (0, 1),
      (1, -3), (1, -2), (1, -1), (1, 1), (1, 2), (1, 3)]
NPAIR = len(NZ)
LAW = 40                       # xh la window: 4 halo + 32 + 4 halo
REPS = 1                       # timing builds set >1 to amortize dispatch

_CACHE = {}


def _host_prep(weight, psi_vals, k_idx, ho_idx, lat_in, lon_in):
    dla_all = lat_in.astype(np.int64) - ho_idx.astype(np.int64)
    P = np.zeros((K, 9, NLAT, NLON), dtype=np.float64)
    np.add.at(P, (k_idx, dla_all + 4, ho_idx, lon_in), psi_vals.astype(np.float64))
    f = np.arange(NF)
    ang = 2 * np.pi * np.outer(np.arange(NLON), f) / NLON          # [360,181]
    dfwd = np.concatenate([np.cos(ang), -np.sin(ang)], axis=1)     # [360,362]
    cf = np.full(NF, 2.0 / NLON)
    cf[0] = 1.0 / NLON
    cf[NF - 1] = 1.0 / NLON
    dinv = np.concatenate([cf[:, None] * np.cos(ang.T),
                           -cf[:, None] * np.sin(ang.T)], axis=0)  # [362,360]
    dinv[NF, :] = 0.0
    dinv[2 * NF - 1, :] = 0.0
    phat = np.zeros((FDIM, NPAIR, PHW), dtype=np.float64)
    for ip, (k, dla) in enumerate(NZ):
        pT = (P[k, dla + 4] @ np.cos(ang)).T                       # [181f, 181ho]
        phat[:NF, ip, :NLAT] = pT
        phat[NF:2 * NF, ip, :NLAT] = pT
    return (np.ascontiguousarray(dfwd.astype(BF16)),
            np.ascontiguousarray(dinv.astype(BF16)),
            np.ascontiguousarray(phat.astype(BF16)))


def _build_nc():
    import os
    import concourse.bass as bass
    import concourse.bacc as bacc
    import concourse.tile as tile
    from concourse import mybir

    skip_ab = os.environ.get("KSKIP_AB") == "1"
    skip_d = os.environ.get("KSKIP_D") == "1"
    skip_e = os.environ.get("KSKIP_E") == "1"
    pe_add = os.environ.get("KD_PEADD", "1") == "1"   # pair-adds on TensorE
    n_gp = int(os.environ.get("KD_GP", "0"))          # pairs mul'd on GPSIMD

    f32 = mybir.dt.float32
    bf = mybir.dt.bfloat16

    nc = bacc.Bacc("TRN2", target_bir_lowering=False, debug=False)

    x_in = nc.dram_tensor("x_in", [CIN, NLAT, NLON], bf, kind="ExternalInput").ap()
    w2_in = nc.dram_tensor("w2_in", [CIN, K * OH], bf, kind="ExternalInput").ap()
    dfwd_in = nc.dram_tensor("dfwd_in", [NLON, FDIM], bf, kind="ExternalInput").ap()
    dinv_in = nc.dram_tensor("dinv_in", [FDIM, NLON], bf, kind="ExternalInput").ap()
    phat_in = nc.dram_tensor("phat_in", [FDIM, NPAIR, PHW], bf, kind="ExternalInput").ap()
    ident_in = nc.dram_tensor("ident_in", [128, 128], bf, kind="ExternalInput").ap()
    # out[band, po, o(16)*3chunks, 32] -> flattened [6*360, 3, 16, 32]
    out_d = nc.dram_tensor("out", [len(BANDS) * NLON, 3, 16, 32], f32,
                           kind="ExternalOutput").ap()

    from contextlib import ExitStack
    with tile.TileContext(nc) as tc, ExitStack() as es:
        consts = es.enter_context(tc.tile_pool(name="consts", bufs=1))
        xh_pool = es.enter_context(tc.tile_pool(name="xh", bufs=2))
        yh_pool = es.enter_context(tc.tile_pool(name="yh", bufs=2))
        ph_pool = es.enter_context(tc.tile_pool(name="ph", bufs=2))
        small = es.enter_context(tc.tile_pool(name="small", bufs=3))
        tmp_pool = es.enter_context(tc.tile_pool(name="tmp", bufs=12))
        stg_pool = es.enter_context(tc.tile_pool(name="stg", bufs=3))
        ps_a = es.enter_context(tc.tile_pool(name="ps_a", bufs=1, space=bass.MemorySpace.PSUM))
        ps_b = es.enter_context(tc.tile_pool(name="ps_b", bufs=2, space=bass.MemorySpace.PSUM))
        ps_e = es.enter_context(tc.tile_pool(name="ps_e", bufs=2, space=bass.MemorySpace.PSUM))
        ps_y = (es.enter_context(tc.tile_pool(name="ps_y", bufs=1, space=bass.MemorySpace.PSUM))
                if pe_add else None)

        w2_sb = consts.tile([CIN, K * OH], bf)
        nc.sync.dma_start(out=w2_sb[:, :], in_=w2_in[:, :])
        dfwd_sb = consts.tile([128, 3, FDIM], bf)
        for j, (p0, p1) in enumerate(PS):
            nc.sync.dma_start(out=dfwd_sb[:p1 - p0, j, :], in_=dfwd_in[p0:p1, :])
        dinv_sb = consts.tile([128, 3, NLON], bf)
        for t, (f0, f1) in enumerate(FS):
            nc.sync.dma_start(out=dinv_sb[:f1 - f0, t, :], in_=dinv_in[f0:f1, :])
        ident_sb = consts.tile([128, 128], bf)
        nc.sync.dma_start(out=ident_sb[:, :], in_=ident_in[:, :])

        for _rep in range(REPS):
          for bi, (ho_lo, ho_hi) in enumerate(BANDS):
            how = ho_hi - ho_lo
            W = (how + 1) & ~1             # even op width (32,32,32,32,32,26)
            la_lo = ho_lo - 4              # virtual (may be <0)
            v0 = max(0, la_lo)             # first valid la
            v1 = min(NLAT, ho_lo + 36)     # last+1 valid la
            c0 = v0 - la_lo                # first valid col in xh
            nla_v = v1 - v0

            phat_sb = ph_pool.tile([128, 3, NPAIR, 32], bf, tag="phat")
            for t, (f0, f1) in enumerate(FS):
                nc.sync.dma_start(
                    out=phat_sb[:f1 - f0, t, :, :W],
                    in_=phat_in[f0:f1, :, ho_lo:ho_lo + W])

            xh0 = xh_pool.tile([128, 3, K * OH, LAW], bf, tag="xh0")
            xh1 = xh_pool.tile([128, 3, K * OH, LAW], bf, tag="xh1")
            if c0 > 0:
                nc.vector.memset(xh0[:, :, :, 0:c0], 0.0)
            if c0 + nla_v < LAW:
                nc.vector.memset(xh0[:, :, :, c0 + nla_v:LAW], 0.0)

            # ---- stages A+B: einsum-T then forward DFT, in 5-la groups ----
            for g in range(0, 0 if skip_ab else nla_v, 5):
                ng = min(5, nla_v - g)
                la0 = v0 + g
                x_t = small.tile([CIN, 5, NLON], bf, tag="x_t")
                nc.sync.dma_start(out=x_t[:, :ng, :], in_=x_in[:, la0:la0 + ng, :])
                xwT = small.tile([128, 3, 5, K * OH], bf, tag="xwT")
                for j, (p0, p1) in enumerate(PS):
                    pc = p1 - p0
                    ps_t = ps_a.tile([128, 5, K * OH], f32, tag="ps_a")
                    for il in range(ng):
                        nc.tensor.matmul(
                            ps_t[:pc, il, :],
                            x_t[:, il, p0:p1],
                            w2_sb[:, :],
                            start=True, stop=True)
                    nc.scalar.copy(xwT[:pc, j, :ng, :], ps_t[:pc, :ng, :])
                for t, (f0, f1) in enumerate(FS):
                    fsz = f1 - f0
                    ps_f = ps_b.tile([128, 5, K * OH], f32, tag="ps_b")
                    for j, (p0, p1) in enumerate(PS):
                        pc = p1 - p0
                        nc.tensor.matmul(
                            ps_f[:fsz, :ng, :],
                            dfwd_sb[:pc, j, f0:f1],
                            xwT[:pc, j, :ng, :],
                            start=(j == 0), stop=(j == 2))
                    # transpose-copy psum [f, la, ok] -> xh0 [f, ok, la]
                    nc.scalar.copy(
                        xh0[:fsz, t, :, c0 + g:c0 + g + ng],
                        ps_f[:fsz, :ng, :].transpose([0, 2, 1]))

            # xh1[c] = xh0[c+1] (parity-aligned copy for odd-dla pairs)
            if skip_ab:
                nc.vector.memset(xh0[:, :, :, :], 0.0)
            nc.gpsimd.tensor_copy(xh1[:, :, :, 0:LAW - 1], xh0[:, :, :, 1:LAW])

            # ---- stage D: spectral multiply-accumulate ----
            # muls on DVE (bf16 2x; a few pairs on GPSIMD); pair-sum via
            # identity-matmul accumulation into a 3-bank fp32 PSUM tile.
            yh = yh_pool.tile([128, 3, OH, 32], bf, tag="yh")
            if skip_d:
                nc.vector.memset(yh[:, :, :, :], 0.0)
            gp_pairs = {(1, -2), (1, 2)} if n_gp == 2 else (
                {(1, -2)} if n_gp == 1 else set())
            for t, (f0, f1) in enumerate([] if skip_d else FS):
                fsz = f1 - f0
                yps = (ps_y.tile([128, OH, 32], f32, tag="ps_y", name="yps")
                       if pe_add else None)
                tms = []
                for ip, (k, dla) in enumerate(NZ):
                    par = (dla + 4) % 2
                    src = xh0 if par == 0 else xh1
                    a = dla + 4 - par
                    xs = src[:fsz, t, OH * k:OH * k + OH, a:a + W]
                    pb = phat_sb[:fsz, t, ip, 0:W]
                    pbc = bass.AP(
                        tensor=pb.tensor, offset=pb.offset,
                        ap=[list(pb.ap[0]), [0, OH], list(pb.ap[1])])
                    if pe_add:
                        tm = tmp_pool.tile([128, OH, 32], bf, tag="tmp")
                        eng = nc.gpsimd if (k, dla) in gp_pairs else nc.vector
                        eng.tensor_mul(tm[:fsz, :, :W], xs, pbc)
                        tms.append(tm)
                    elif ip == 0:
                        if W < 32:
                            nc.vector.memset(yh[:fsz, t, :, W:32], 0.0)
                        nc.vector.tensor_mul(yh[:fsz, t, :, :W], xs, pbc)
                    else:
                        tm = tmp_pool.tile([128, OH, 32], bf, tag="tmp")
                        nc.vector.tensor_mul(tm[:fsz, :, :W], xs, pbc)
                        nc.vector.tensor_add(
                            yh[:fsz, t, :, :W], yh[:fsz, t, :, :W],
                            tm[:fsz, :, :W])
                if pe_add:
                    for ip, tm in enumerate(tms):
                        for c in range(3):
                            nc.tensor.matmul(
                                yps[:fsz, 16 * c:16 * c + 16, :],
                                ident_sb[:fsz, :fsz],
                                tm[:fsz, 16 * c:16 * c + 16, :],
                                start=(ip == 0), stop=(ip == NPAIR - 1))
                    nc.scalar.copy(yh[:fsz, t, :, :], yps[:fsz, :, :])

            # ---- stage E: inverse DFT (dinv stationary) + store ----
            for j, (p0, p1) in enumerate([] if skip_e else PS):
                pc = p1 - p0
                for oc in range(3):
                    ps_o = ps_e.tile([128, 16, 32], f32, tag="ps_e")
                    for t, (f0, f1) in enumerate(FS):
                        fsz = f1 - f0
                        nc.tensor.matmul(
                            ps_o[:pc, :, :],
                            dinv_sb[:fsz, t, p0:p1],
                            yh[:fsz, t, 16 * oc:16 * oc + 16, :],
                            start=(t == 0), stop=(t == 2))
                    o_sb = stg_pool.tile([128, 16, 32], f32, tag="o_sb")
                    nc.scalar.copy(o_sb[:pc, :, :], ps_o[:pc, :, :])
                    nc.sync.dma_start(
                        out=out_d[bi * NLON + p0:bi * NLON + p1, oc, :, :],
                        in_=o_sb[:pc, :, :])

    nc.compile()
    return nc


def _get_runner(n_cores=8):
    """Build (once) a jitted shard_map runner for the compiled Bass module."""
    if "runner" in _CACHE:
        return _CACHE["runner"]
    import jax
    import jax.numpy as jnp
    from jax.sharding import Mesh, PartitionSpec, NamedSharding
    from jax.experimental.shard_map import shard_map
    from concourse import bass2jax, mybir

    if "nc" not in _CACHE:
        _CACHE["nc"] = _build_nc()
    nc = _CACHE["nc"]
    bass2jax.install_neuronx_cc_hook()

    partition_name = (nc.partition_id_tensor.name
                      if nc.partition_id_tensor else None)
    in_names, out_names, out_avals = [], [], []
    for alloc in nc.m.functions[0].allocations:
        if not isinstance(alloc, mybir.MemoryLocationSet):
            continue
        name = alloc.memorylocations[0].name
        if alloc.kind == "ExternalInput":
            if name != partition_name:
                in_names.append(name)
        elif alloc.kind == "ExternalOutput":
            out_names.append(name)
            out_avals.append(jax.core.ShapedArray(
                tuple(alloc.tensor_shape), mybir.dt.np(alloc.dtype)))
    n_params = len(in_names)
    n_outs = len(out_avals)
    all_names = in_names + out_names
    if partition_name is not None:
        all_names = all_names + [partition_name]

    def _body(*args):
        operands = list(args)
        if partition_name is not None:
            operands.append(bass2jax.partition_id_tensor())
        outs = bass2jax._bass_exec_p.bind(
            *operands,
            out_avals=tuple(out_avals),
            in_names=tuple(all_names),
            out_names=tuple(out_names),
            lowering_input_output_aliases=(),
            sim_require_finite=True,
            sim_require_nnan=True,
            nc=nc,
        )
        return tuple(outs)

    devices = jax.devices()[:n_cores]
    mesh = Mesh(np.asarray(devices), ("core",))
    spec = PartitionSpec("core")
    sharding = NamedSharding(mesh, spec)
    donate = tuple(range(n_params, n_params + n_outs))
    sharded = jax.jit(
        shard_map(_body, mesh=mesh, in_specs=(spec,) * (n_params + n_outs),
                  out_specs=(spec,) * n_outs, check_rep=False),
        donate_argnums=donate, keep_unused=True)
    zero_shapes = [(n_cores * a.shape[0], *a.shape[1:]) for a in out_avals]
    zero_dtypes = [a.dtype for a in out_avals]
    make_zeros = jax.jit(
        lambda: tuple(jnp.zeros(s, d) for s, d in zip(zero_shapes, zero_dtypes)),
        out_shardings=(sharding,) * n_outs)
    runner = {
        "sharded": sharded, "make_zeros": make_zeros, "sharding": sharding,
        "in_names": in_names, "out_names": out_names, "out_avals": out_avals,
        "n_cores": n_cores,
    }
    _CACHE["runner"] = runner
    return runner


def _get_chain_runner(n_chain):
    """Jitted runner executing the NEFF n_chain times with output-chaining
    (each iteration consumes the previous outputs as its donated out-buffers)
    so XLA cannot CSE the repeats.  Used only for timing."""
    key = ("chain", n_chain)
    if key in _CACHE:
        return _CACHE[key]
    import jax
    from jax.sharding import Mesh, PartitionSpec
    from jax.experimental.shard_map import shard_map
    from concourse import bass2jax

    runner = _get_runner()
    nc = _CACHE["nc"]
    n_params = len(runner["in_names"])
    n_outs = len(runner["out_names"])
    out_avals = runner["out_avals"]
    partition_name = (nc.partition_id_tensor.name
                      if nc.partition_id_tensor else None)
    all_names = runner["in_names"] + runner["out_names"]
    if partition_name is not None:
        all_names = all_names + [partition_name]

    def _body(*args):
        params = list(args[:n_params])
        outs = list(args[n_params:])
        for _ in range(n_chain):
            operands = params + outs
            if partition_name is not None:
                operands.append(bass2jax.partition_id_tensor())
            outs = list(bass2jax._bass_exec_p.bind(
                *operands,
                out_avals=tuple(out_avals),
                in_names=tuple(all_names),
                out_names=tuple(runner["out_names"]),
                lowering_input_output_aliases=(),
                sim_require_finite=True,
                sim_require_nnan=True,
                nc=nc,
            ))
        return tuple(outs)

    devices = jax.devices()[:runner["n_cores"]]
    mesh = Mesh(np.asarray(devices), ("core",))
    spec = PartitionSpec("core")
    fn = jax.jit(
        shard_map(_body, mesh=mesh, in_specs=(spec,) * (n_params + n_outs),
                  out_specs=(spec,) * n_outs, check_rep=False),
        donate_argnums=tuple(range(n_params, n_params + n_outs)),
        keep_unused=True)
    _CACHE[key] = fn
    return fn


def _device_inputs(x, weight, psi_arrays):
    """Concatenated-global per-parameter arrays, device_put with sharding."""
    import jax
    dfwd, dinv, phat = _host_prep(weight, *psi_arrays)
    x_bf = np.ascontiguousarray(x.astype(BF16))
    ident = np.eye(128, dtype=BF16)
    per_core = {"x_in": [], "w2_in": [], "dfwd_in": [], "dinv_in": [],
                "phat_in": [], "ident_in": []}
    for s in range(8):
        b, ohf = s // 2, s % 2
        o_sl = slice(OH * ohf, OH * ohf + OH)
        # w2[c, k*48+o] = weight[o_global, c, k]
        w2 = np.ascontiguousarray(
            weight[o_sl].transpose(1, 2, 0).reshape(CIN, K * OH).astype(BF16))
        per_core["x_in"].append(x_bf[b])
        per_core["w2_in"].append(w2)
        per_core["dfwd_in"].append(dfwd)
        per_core["dinv_in"].append(dinv)
        per_core["phat_in"].append(phat)
        per_core["ident_in"].append(ident)
    runner = _get_runner()
    concat = {k: np.concatenate(v, axis=0) for k, v in per_core.items()}
    return [jax.device_put(concat[name], runner["sharding"])
            for name in runner["in_names"]]


def _run_device(dev_in):
    runner = _get_runner()
    zeros = runner["make_zeros"]()
    return runner["sharded"](*dev_in, *zeros)


def kernel(x, weight, bias, psi_vals, k_idx, ho_idx, lat_in_idx, lon_in_idx):
    x = np.ascontiguousarray(np.asarray(x, dtype=np.float32))
    weight = np.asarray(weight, dtype=np.float32)
    bias = np.asarray(bias, dtype=np.float32)
    psi_arrays = (np.asarray(psi_vals), np.asarray(k_idx), np.asarray(ho_idx),
                  np.asarray(lat_in_idx), np.asarray(lon_in_idx))

    dev_in = _device_inputs(x, weight, psi_arrays)
    out_arrs = _run_device(dev_in)
    runner = _get_runner()
    a0 = runner["out_avals"][0]
    res0 = np.asarray(out_arrs[0]).reshape(8, *a0.shape)

    out = np.empty((B, COUT, NLAT, NLON), dtype=np.float32)
    for s in range(8):
        b, ohf = s // 2, s % 2
        # res0[s]: [6*360, 3, 16, 32] -> [6, 360, 48, 32]
        arr = res0[s].reshape(len(BANDS), NLON, 48, 32)
        for bi, (ho_lo, ho_hi) in enumerate(BANDS):
            how = ho_hi - ho_lo
            out[b, OH * ohf:OH * ohf + OH, ho_lo:ho_hi, :] = (
                arr[bi, :, :, :how].transpose(1, 2, 0))
    if np.any(bias):
        out += bias[None, :, None, None]
    return out
